# revision 2
# baseline (speedup 1.0000x reference)
"""Trainium2 Bass kernel for a 2-layer edge-weighted GraphSAGE network.

Strategy (8 NeuronCores, dst-sharded):
  * Host converts the edge list (src, dst, w) into the dense row-normalized
    adjacency operator A[d, s] = sum_e w_e / max(deg_d, 1), so each layer's
    weighted segment-mean becomes a dense matmul h_N = A @ h.
  * Nodes (rows of A) are sharded across the 8 cores: core c owns dst range
    [1250c, 1250(c+1)).  Each core streams its A^T shard from HBM in
    [128 src, 1250 dst] fp16 blocks and accumulates
    h_N^T[f, d] += h_k^T . A^T_k on the TensorEngine (features on PSUM
    partitions, local dst nodes on the free axis).
  * The small linear layers run in the same transposed layout; ReLU+bias on
    the ScalarEngine; layer-1 output is PE-transposed back to row-major and
    AllGathered so every core has the full x for layer 2's gather side.
  * All feature/operator tensors are fp16 (measured end-to-end rel-err vs the
    f32 reference: ~4e-4); PSUM accumulation is f32; final output is f32.
"""

import os
import sys
import types

sys.path.insert(0, "/opt/trn_rl_repo")

import numpy as np

import concourse.bacc as bacc
import concourse.tile as tile
from concourse import mybir
from concourse import bass_utils
from concourse.masks import make_identity

N_NODES = 10000
N_EDGES = 640000
D_IN, D_HID, D_OUT = 128, 256, 64
N_CORES = 8
P = 128
NB = N_NODES // N_CORES          # 1250 local dst nodes per core
KB = (N_NODES + P - 1) // P      # 79 src k-blocks
NPAD = KB * P                    # 10112
F16 = mybir.dt.float16
F32 = mybir.dt.float32

# free-axis chunks of the local dst range (PSUM bank = 512 f32)
N_CHUNKS = [(0, 512), (512, 1024), (1024, NB)]
DST_BLOCKS = [(b * P, min((b + 1) * P, NB)) for b in range((NB + P - 1) // P)]

_compiled_nc = None
LAST_EXEC_NS = None


def _build_nc():
    nc = bacc.Bacc("TRN2", target_bir_lowering=False, debug=False,
                   num_devices=N_CORES)

    at_d = nc.dram_tensor("at", [KB, P, NB], F16, kind="ExternalInput")
    hk_d = nc.dram_tensor("hk", [KB, P, D_IN], F16, kind="ExternalInput")
    ht_d = nc.dram_tensor("ht", [D_IN, NB], F16, kind="ExternalInput")
    w1_d = nc.dram_tensor("w1", [2 * D_IN, D_HID], F16, kind="ExternalInput")
    w2_d = nc.dram_tensor("w2", [2 * D_HID, D_OUT], F16, kind="ExternalInput")
    b1_d = nc.dram_tensor("b1c", [P, 2], F32, kind="ExternalInput")
    b2_d = nc.dram_tensor("b2c", [D_OUT, 1], F32, kind="ExternalInput")
    out_d = nc.dram_tensor("outT", [D_OUT, NB], F32, kind="ExternalOutput")

    with tile.TileContext(nc) as tc:
        with (
            tc.tile_pool(name="const", bufs=1) as cpool,
            tc.tile_pool(name="work", bufs=1) as wpool,
            tc.tile_pool(name="astream", bufs=4) as apool,
            tc.tile_pool(name="dram", bufs=1, space="DRAM") as dpool,
        ):
            # ---- constant loads -------------------------------------------------
            hks = cpool.tile([P, KB * D_IN], F16)   # h k-blocks, block k at cols [k*128, k*128+128)
            nc.sync.dma_start(
                out=hks[:].rearrange("p (k f) -> p k f", f=D_IN),
                in_=hk_d[:].rearrange("k p f -> p k f"),
            )
            hts = cpool.tile([P, NB], F16)
            nc.sync.dma_start(out=hts[:], in_=ht_d[:])
            w1s = cpool.tile([P, 2 * D_HID], F16)   # k-block k at cols [k*256, k*256+256)
            for k in range(2):
                nc.sync.dma_start(out=w1s[:, k * D_HID:(k + 1) * D_HID],
                                  in_=w1_d[k * P:(k + 1) * P, :])
            w2s = cpool.tile([P, 4 * D_OUT], F16)   # k-block k at cols [k*64, k*64+64)
            for k in range(4):
                nc.sync.dma_start(out=w2s[:, k * D_OUT:(k + 1) * D_OUT],
                                  in_=w2_d[k * P:(k + 1) * P, :])
            b1s = cpool.tile([P, 2], F32)
            nc.sync.dma_start(out=b1s[:], in_=b1_d[:])
            b2s = cpool.tile([D_OUT, 1], F32)
            nc.sync.dma_start(out=b2s[:], in_=b2_d[:])
            ident = cpool.tile([P, P], F16)
            make_identity(nc, ident[:])

            hNT = wpool.tile([P, NB], F16)
            xT = [wpool.tile([P, NB], F16, name=f"xT{m}") for m in range(2)]
            xNT = [wpool.tile([P, NB], F16, name=f"xNT{m}") for m in range(2)]
            xloc = wpool.tile([P, len(DST_BLOCKS) * D_HID], F16)
            xs = wpool.tile([P, KB * D_HID], F16)   # full x, k-block k at [k*256, k*256+256)
            outsb = wpool.tile([D_OUT, NB], F32)

            # ---- layer 1 aggregation: hN^T = sum_k hk^T . A^T_k ----------------
            with tc.tile_pool(name="ps1", bufs=1, space="PSUM") as ps1:
                hN_ps = ps1.tile([P, NB], F32, space="PSUM")
                for k in range(KB):
                    a_t = apool.tile([P, NB], F16, name="a_t")
                    nc.sync.dma_start(out=a_t[:], in_=at_d[k])
                    for (n0, n1) in N_CHUNKS:
                        nc.tensor.matmul(out=hN_ps[:, n0:n1],
                                         lhsT=hks[:, k * D_IN:(k + 1) * D_IN],
                                         rhs=a_t[:, n0:n1],
                                         start=(k == 0), stop=(k == KB - 1))
                nc.scalar.activation(out=hNT[:], in_=hN_ps[:],
                                     func=mybir.ActivationFunctionType.Copy)

            # ---- layer 1 linear: x^T = relu(W1^T . [h; hN]^T + b1) -------------
            cat1 = [hts, hNT]
            with tc.tile_pool(name="ps2", bufs=1, space="PSUM") as ps2:
                y_ps = [ps2.tile([P, NB], F32, space="PSUM", name=f"y_ps{m}")
                        for m in range(2)]
                for m in range(2):
                    for (n0, n1) in N_CHUNKS:
                        for k in range(2):
                            nc.tensor.matmul(
                                out=y_ps[m][:, n0:n1],
                                lhsT=w1s[:, k * D_HID + m * P: k * D_HID + (m + 1) * P],
                                rhs=cat1[k][:, n0:n1],
                                start=(k == 0), stop=(k == 1))
                for m in range(2):
                    nc.scalar.activation(out=xT[m][:], in_=y_ps[m][:],
                                         func=mybir.ActivationFunctionType.Relu,
                                         bias=b1s[:, m:m + 1])

            # ---- transpose x^T -> x (row-major local shard) --------------------
            with tc.tile_pool(name="ps3", bufs=2, space="PSUM") as ps3:
                for b, (d0, d1) in enumerate(DST_BLOCKS):
                    bw = d1 - d0
                    for m in range(2):
                        tps = ps3.tile([P, P], F16, space="PSUM", name="tps")
                        nc.tensor.transpose(out=tps[:bw, :],
                                            in_=xT[m][:, d0:d1],
                                            identity=ident[:])
                        nc.vector.tensor_copy(
                            out=xloc[:bw, b * D_HID + m * P: b * D_HID + (m + 1) * P],
                            in_=tps[:bw, :])

            # ---- all-gather x across cores -------------------------------------
            ag_in = dpool.tile([NB, D_HID], F16)
            ag_out = dpool.tile([N_NODES, D_HID], F16, addr_space="Shared")
            for b, (d0, d1) in enumerate(DST_BLOCKS):
                bw = d1 - d0
                nc.sync.dma_start(out=ag_in[d0:d1, :],
                                  in_=xloc[:bw, b * D_HID:(b + 1) * D_HID])
            nc.gpsimd.collective_compute(
                "AllGather", mybir.AluOpType.bypass,
                replica_groups=[list(range(N_CORES))],
                ins=[ag_in.opt()], outs=[ag_out.opt()])

            # ---- load full x into SBUF k-blocks --------------------------------
            full_rows = (KB - 1) * P   # 9984
            nc.sync.dma_start(
                out=xs[:, :full_rows * 2].rearrange("p (k f) -> p k f", f=D_HID),
                in_=ag_out[:full_rows, :].rearrange("(k p) f -> p k f", p=P))
            nc.vector.memset(xs[:, (KB - 1) * D_HID:], 0.0)
            tail = N_NODES - full_rows  # 16
            nc.sync.dma_start(out=xs[:tail, (KB - 1) * D_HID: KB * D_HID],
                              in_=ag_out[full_rows:, :])

            # ---- layer 2 aggregation: xN^T = sum_k xk^T . A^T_k ----------------
            with tc.tile_pool(name="ps4", bufs=1, space="PSUM") as ps4:
                xN_ps = [ps4.tile([P, NB], F32, space="PSUM", name=f"xN_ps{m}")
                         for m in range(2)]
                for k in range(KB):
                    a_t2 = apool.tile([P, NB], F16, name="a_t2")
                    nc.sync.dma_start(out=a_t2[:], in_=at_d[k])
                    for m in range(2):
                        for (n0, n1) in N_CHUNKS:
                            nc.tensor.matmul(
                                out=xN_ps[m][:, n0:n1],
                                lhsT=xs[:, k * D_HID + m * P: k * D_HID + (m + 1) * P],
                                rhs=a_t2[:, n0:n1],
                                start=(k == 0), stop=(k == KB - 1))
                for m in range(2):
                    nc.scalar.activation(out=xNT[m][:], in_=xN_ps[m][:],
                                         func=mybir.ActivationFunctionType.Copy)

            # ---- layer 2 linear: out^T = W2^T . [x; xN]^T + b2 -----------------
            cat2 = [xT[0], xT[1], xNT[0], xNT[1]]
            with tc.tile_pool(name="ps5", bufs=1, space="PSUM") as ps5:
                o_ps = ps5.tile([D_OUT, NB], F32, space="PSUM")
                for (n0, n1) in N_CHUNKS:
                    for k in range(4):
                        nc.tensor.matmul(
                            out=o_ps[:, n0:n1],
                            lhsT=w2s[:, k * D_OUT:(k + 1) * D_OUT],
                            rhs=cat2[k][:, n0:n1],
                            start=(k == 0), stop=(k == 3))
                nc.scalar.activation(out=outsb[:], in_=o_ps[:],
                                     func=mybir.ActivationFunctionType.Identity,
                                     bias=b2s[:, 0:1])
            nc.sync.dma_start(out=out_d[:], in_=outsb[:])

    nc.compile()
    return nc


def _get_nc():
    global _compiled_nc
    if _compiled_nc is None:
        _compiled_nc = _build_nc()
    return _compiled_nc


def _enable_profile_hook():
    """Register the NTFF profiling hook that trn_boot skips when the image's
    antenv lacks axon_hooks (profiling only; used when GNN_PROFILE=1)."""
    try:
        import antenv
        if "antenv.axon_hooks" not in sys.modules:
            mod = types.ModuleType("antenv.axon_hooks")
            _h = [None]
            mod.set_axon_ntff_profile_hook = lambda hook: _h.__setitem__(0, hook)
            mod.get_axon_ntff_profile_hook = lambda: _h[0]
            sys.modules["antenv.axon_hooks"] = mod
            antenv.axon_hooks = mod
        from trn_agent_boot.trn_boot import _ntff_profile_via_ctypes
        hook = _ntff_profile_via_ctypes("/opt/axon/libaxon_pjrt.so")
        if hook is not None:
            sys.modules["antenv.axon_hooks"].set_axon_ntff_profile_hook(hook)
            return True
    except Exception:
        pass
    return False


def kernel(h, w, src, dst, W1, b1, W2, b2):
    global LAST_EXEC_NS
    h = np.asarray(h, dtype=np.float32)
    w = np.asarray(w, dtype=np.float32)
    src = np.asarray(src)
    dst = np.asarray(dst)
    W1 = np.asarray(W1, dtype=np.float32)
    b1 = np.asarray(b1, dtype=np.float32)
    W2 = np.asarray(W2, dtype=np.float32)
    b2 = np.asarray(b2, dtype=np.float32)

    # ---- host: edge list -> dense normalized operator, sharded ------------
    import scipy.sparse as sp
    deg = np.bincount(dst, minlength=N_NODES).astype(np.float32)
    w_norm = (w[:, 0] / np.maximum(deg, 1.0)[dst]).astype(np.float32)
    # AT[s, d] = sum of w_norm over edges (s -> d): A^T built directly
    AT = sp.coo_matrix((w_norm, (src, dst)), shape=(N_NODES, N_NODES)).toarray()
    AT16 = np.zeros((NPAD, N_NODES), dtype=np.float16)
    AT16[:N_NODES] = AT
    hp = np.zeros((NPAD, D_IN), dtype=np.float16)
    hp[:N_NODES] = h
    hk = np.ascontiguousarray(hp.reshape(KB, P, D_IN))
    w1c = W1.astype(np.float16)
    w2c = W2.astype(np.float16)
    b1c = np.ascontiguousarray(b1.reshape(2, P).T)
    b2c = b2.reshape(D_OUT, 1)

    in_maps = []
    for c in range(N_CORES):
        sl = slice(c * NB, (c + 1) * NB)
        in_maps.append({
            "at": np.ascontiguousarray(AT16[:, sl].reshape(KB, P, NB)),
            "hk": hk,
            "ht": np.ascontiguousarray(h[sl].T.astype(np.float16)),
            "w1": w1c,
            "w2": w2c,
            "b1c": b1c,
            "b2c": b2c,
        })

    nc = _get_nc()
    trace = os.environ.get("GNN_PROFILE") == "1" and _enable_profile_hook()
    res = bass_utils.run_bass_kernel_spmd(
        nc, in_maps, core_ids=list(range(N_CORES)), trace=trace)
    LAST_EXEC_NS = res.exec_time_ns

    out = np.concatenate(
        [res.results[c]["outT"].T for c in range(N_CORES)], axis=0)
    return out.astype(np.float32)


# revision 3
# speedup vs baseline: 1.1884x; 1.1884x over previous
"""Trainium2 Bass kernel for a 2-layer edge-weighted GraphSAGE network.

Strategy (8 NeuronCores, dst-sharded):
  * Host converts the edge list (src, dst, w) into the dense row-normalized
    adjacency operator A[d, s] = sum_e w_e / max(deg_d, 1), so each layer's
    weighted segment-mean becomes a dense matmul h_N = A @ h.
  * Nodes (rows of A) are sharded across the 8 cores: core c owns dst range
    [1250c, 1250(c+1)).  Each core streams its A^T shard from HBM in
    [128 src, 1250 dst] fp16 blocks (fused with the matching h k-block so
    layer 1 is a single contiguous stream) and accumulates
    h_N^T[f, d] += h_k^T . A^T_k on the TensorEngine (features on PSUM
    partitions, local dst nodes on the free axis).
  * The first A_CACHE k-blocks stay resident in SBUF and are reused by
    layer 2's aggregation, cutting the second pass over A.
  * The small linear layers run in the same transposed layout; ReLU+bias on
    the ScalarEngine; layer-1 output is PE-transposed back to row-major and
    AllGathered so every core has the full x for layer 2's gather side.
  * All feature/operator tensors are fp16 (measured end-to-end rel-err vs the
    f32 reference: ~4e-4); PSUM accumulation is f32; final output is f32.
"""

import os
import sys
import types

sys.path.insert(0, "/opt/trn_rl_repo")

import numpy as np

import concourse.bacc as bacc
import concourse.tile as tile
from concourse import mybir
from concourse import bass_utils
from concourse.masks import make_identity

N_NODES = 10000
N_EDGES = 640000
D_IN, D_HID, D_OUT = 128, 256, 64
N_CORES = 8
P = 128
NB = N_NODES // N_CORES          # 1250 local dst nodes per core
KB = (N_NODES + P - 1) // P      # 79 src k-blocks
NPAD = KB * P                    # 10112
FW = D_IN + NB                   # fused stream row width: h block | A^T block
A_CACHE = 42                     # k-blocks of the fused stream kept in SBUF
F16 = mybir.dt.float16
F32 = mybir.dt.float32

# free-axis chunks of the local dst range (PSUM bank = 512 f32)
N_CHUNKS = [(0, 512), (512, 1024), (1024, NB)]
DST_BLOCKS = [(b * P, min((b + 1) * P, NB)) for b in range((NB + P - 1) // P)]

_compiled_nc = None
LAST_EXEC_NS = None


def _build_nc():
    nc = bacc.Bacc("TRN2", target_bir_lowering=False, debug=False,
                   num_devices=N_CORES)

    fs_d = nc.dram_tensor("fs", [KB, P, FW], F16, kind="ExternalInput")
    ht_d = nc.dram_tensor("ht", [D_IN, NB], F16, kind="ExternalInput")
    w1_d = nc.dram_tensor("w1", [2 * D_IN, D_HID], F16, kind="ExternalInput")
    w2_d = nc.dram_tensor("w2", [2 * D_HID, D_OUT], F16, kind="ExternalInput")
    b1_d = nc.dram_tensor("b1c", [P, 2], F32, kind="ExternalInput")
    b2_d = nc.dram_tensor("b2c", [D_OUT, 1], F32, kind="ExternalInput")
    out_d = nc.dram_tensor("outT", [D_OUT, NB], F32, kind="ExternalOutput")

    with tile.TileContext(nc) as tc:
        with (
            tc.tile_pool(name="const", bufs=1) as cpool,
            tc.tile_pool(name="acache", bufs=1) as acpool,
            tc.tile_pool(name="work", bufs=1) as wpool,
            tc.tile_pool(name="astream", bufs=1) as apool,
            tc.tile_pool(name="xstream", bufs=1) as xpool,
            tc.tile_pool(name="dram", bufs=1, space="DRAM") as dpool,
        ):
            # ---- constant loads (ACT HWDGE ring; A-stream owns the SP ring) ----
            hts = cpool.tile([P, NB], F16)
            nc.scalar.dma_start(out=hts[:], in_=ht_d[:])
            w1s = cpool.tile([P, 2 * D_HID], F16)   # k-block k at cols [k*256, k*256+256)
            for k in range(2):
                nc.scalar.dma_start(out=w1s[:, k * D_HID:(k + 1) * D_HID],
                                    in_=w1_d[k * P:(k + 1) * P, :])
            w2s = cpool.tile([P, 4 * D_OUT], F16)   # k-block k at cols [k*64, k*64+64)
            for k in range(4):
                nc.scalar.dma_start(out=w2s[:, k * D_OUT:(k + 1) * D_OUT],
                                    in_=w2_d[k * P:(k + 1) * P, :])
            b1s = cpool.tile([P, 2], F32)
            nc.scalar.dma_start(out=b1s[:], in_=b1_d[:])
            b2s = cpool.tile([D_OUT, 1], F32)
            nc.scalar.dma_start(out=b2s[:], in_=b2_d[:])
            ident = cpool.tile([P, P], F16)
            make_identity(nc, ident[:])

            hNT = wpool.tile([P, NB], F16)
            xT = [wpool.tile([P, NB], F16, name=f"xT{m}") for m in range(2)]
            xNT = [wpool.tile([P, NB], F16, name=f"xNT{m}") for m in range(2)]
            xloc = wpool.tile([P, len(DST_BLOCKS) * D_HID], F16)
            outsb = wpool.tile([D_OUT, NB], F32)

            # resident cache of the first A_CACHE fused k-blocks (h | A^T)
            acache = [acpool.tile([P, FW], F16, name=f"ac{k}")
                      for k in range(A_CACHE)]
            for k in range(A_CACHE):
                nc.sync.dma_start(out=acache[k][:], in_=fs_d[k])

            # ---- layer 1 aggregation: hN^T = sum_k hk^T . A^T_k ----------------
            with tc.tile_pool(name="ps1", bufs=1, space="PSUM") as ps1:
                hN_ps = ps1.tile([P, NB], F32, space="PSUM")
                for k in range(KB):
                    if k < A_CACHE:
                        blk = acache[k]
                    else:
                        blk = apool.tile([P, FW], F16, name="a_t", bufs=6)
                        nc.sync.dma_start(out=blk[:], in_=fs_d[k])
                    for (n0, n1) in N_CHUNKS:
                        nc.tensor.matmul(out=hN_ps[:, n0:n1],
                                         lhsT=blk[:, :D_IN],
                                         rhs=blk[:, D_IN + n0:D_IN + n1],
                                         start=(k == 0), stop=(k == KB - 1))
                nc.scalar.activation(out=hNT[:], in_=hN_ps[:],
                                     func=mybir.ActivationFunctionType.Copy)

            # ---- layer 1 linear: x^T = relu(W1^T . [h; hN]^T + b1) -------------
            cat1 = [hts, hNT]
            with tc.tile_pool(name="ps2", bufs=1, space="PSUM") as ps2:
                y_ps = [ps2.tile([P, NB], F32, space="PSUM", name=f"y_ps{m}")
                        for m in range(2)]
                for m in range(2):
                    for (n0, n1) in N_CHUNKS:
                        for k in range(2):
                            nc.tensor.matmul(
                                out=y_ps[m][:, n0:n1],
                                lhsT=w1s[:, k * D_HID + m * P: k * D_HID + (m + 1) * P],
                                rhs=cat1[k][:, n0:n1],
                                start=(k == 0), stop=(k == 1))
                for m in range(2):
                    nc.scalar.activation(out=xT[m][:], in_=y_ps[m][:],
                                         func=mybir.ActivationFunctionType.Relu,
                                         bias=b1s[:, m:m + 1])

            # ---- transpose x^T -> x (row-major local shard) --------------------
            with tc.tile_pool(name="ps3", bufs=2, space="PSUM") as ps3:
                for b, (d0, d1) in enumerate(DST_BLOCKS):
                    bw = d1 - d0
                    for m in range(2):
                        tps = ps3.tile([P, P], F16, space="PSUM", name="tps")
                        nc.tensor.transpose(out=tps[:bw, :],
                                            in_=xT[m][:, d0:d1],
                                            identity=ident[:])
                        nc.vector.tensor_copy(
                            out=xloc[:bw, b * D_HID + m * P: b * D_HID + (m + 1) * P],
                            in_=tps[:bw, :])

            # ---- all-gather x across cores -------------------------------------
            ag_in = dpool.tile([NB, D_HID], F16)
            ag_out = dpool.tile([N_NODES, D_HID], F16, addr_space="Shared")
            for b, (d0, d1) in enumerate(DST_BLOCKS):
                bw = d1 - d0
                nc.scalar.dma_start(out=ag_in[d0:d1, :],
                                    in_=xloc[:bw, b * D_HID:(b + 1) * D_HID])
            nc.gpsimd.collective_compute(
                "AllGather", mybir.AluOpType.bypass,
                replica_groups=[list(range(N_CORES))],
                ins=[ag_in.opt()], outs=[ag_out.opt()])

            # ---- layer 2 aggregation: xN^T = sum_k xk^T . A^T_k ----------------
            # x k-blocks are streamed straight out of the all-gather buffer;
            # A^T comes from the SBUF cache for k < A_CACHE, else re-streamed.
            with tc.tile_pool(name="ps4", bufs=1, space="PSUM") as ps4:
                xN_ps = [ps4.tile([P, NB], F32, space="PSUM", name=f"xN_ps{m}")
                         for m in range(2)]
                for k in range(KB):
                    xsb = xpool.tile([P, D_HID], F16, name="xsb", bufs=8)
                    r0 = k * P
                    r1 = min((k + 1) * P, N_NODES)
                    if r1 - r0 < P:
                        nc.vector.memset(xsb[:], 0.0)
                    nc.scalar.dma_start(out=xsb[:r1 - r0, :], in_=ag_out[r0:r1, :])
                    if k < A_CACHE:
                        art = acache[k][:, D_IN:]
                    else:
                        a_t2 = apool.tile([P, NB], F16, name="a_t", bufs=6)
                        nc.sync.dma_start(out=a_t2[:], in_=fs_d[k, :, D_IN:])
                        art = a_t2[:]
                    for m in range(2):
                        for (n0, n1) in N_CHUNKS:
                            nc.tensor.matmul(
                                out=xN_ps[m][:, n0:n1],
                                lhsT=xsb[:, m * P:(m + 1) * P],
                                rhs=art[:, n0:n1],
                                start=(k == 0), stop=(k == KB - 1))
                for m in range(2):
                    nc.scalar.activation(out=xNT[m][:], in_=xN_ps[m][:],
                                         func=mybir.ActivationFunctionType.Copy)

            # ---- layer 2 linear: out^T = W2^T . [x; xN]^T + b2 -----------------
            cat2 = [xT[0], xT[1], xNT[0], xNT[1]]
            with tc.tile_pool(name="ps5", bufs=1, space="PSUM") as ps5:
                o_ps = ps5.tile([D_OUT, NB], F32, space="PSUM")
                for (n0, n1) in N_CHUNKS:
                    for k in range(4):
                        nc.tensor.matmul(
                            out=o_ps[:, n0:n1],
                            lhsT=w2s[:, k * D_OUT:(k + 1) * D_OUT],
                            rhs=cat2[k][:, n0:n1],
                            start=(k == 0), stop=(k == 3))
                nc.scalar.activation(out=outsb[:], in_=o_ps[:],
                                     func=mybir.ActivationFunctionType.Identity,
                                     bias=b2s[:, 0:1])
            nc.sync.dma_start(out=out_d[:], in_=outsb[:])

    nc.compile()
    return nc


def _get_nc():
    global _compiled_nc
    if _compiled_nc is None:
        _compiled_nc = _build_nc()
    return _compiled_nc


def _enable_profile_hook():
    """Register the NTFF profiling hook that trn_boot skips when the image's
    antenv lacks axon_hooks (profiling only; used when GNN_PROFILE=1)."""
    try:
        import antenv
        if "antenv.axon_hooks" not in sys.modules:
            mod = types.ModuleType("antenv.axon_hooks")
            _h = [None]
            mod.set_axon_ntff_profile_hook = lambda hook: _h.__setitem__(0, hook)
            mod.get_axon_ntff_profile_hook = lambda: _h[0]
            sys.modules["antenv.axon_hooks"] = mod
            antenv.axon_hooks = mod
        from trn_agent_boot.trn_boot import _ntff_profile_via_ctypes
        hook = _ntff_profile_via_ctypes("/opt/axon/libaxon_pjrt.so")
        if hook is not None:
            sys.modules["antenv.axon_hooks"].set_axon_ntff_profile_hook(hook)
            return True
    except Exception:
        pass
    return False


def _host_prep(h, w, src, dst, W1, b1, W2, b2):
    import scipy.sparse as sp
    deg = np.bincount(dst, minlength=N_NODES).astype(np.float32)
    w_norm = (w[:, 0] / np.maximum(deg, 1.0)[dst]).astype(np.float32)
    # AT[s, d] = sum of w_norm over edges (s -> d): A^T built directly
    AT = sp.coo_matrix((w_norm, (src, dst)), shape=(N_NODES, N_NODES)).toarray()
    AT16 = np.zeros((NPAD, N_NODES), dtype=np.float16)
    AT16[:N_NODES] = AT
    hp = np.zeros((NPAD, D_IN), dtype=np.float16)
    hp[:N_NODES] = h

    w1c = W1.astype(np.float16)
    w2c = W2.astype(np.float16)
    b1c = np.ascontiguousarray(b1.reshape(2, P).T)
    b2c = b2.reshape(D_OUT, 1)

    in_maps = []
    for c in range(N_CORES):
        sl = slice(c * NB, (c + 1) * NB)
        fs = np.empty((KB, P, FW), dtype=np.float16)
        fs[:, :, :D_IN] = hp.reshape(KB, P, D_IN)
        fs[:, :, D_IN:] = AT16[:, sl].reshape(KB, P, NB)
        in_maps.append({
            "fs": fs,
            "ht": np.ascontiguousarray(h[sl].T.astype(np.float16)),
            "w1": w1c,
            "w2": w2c,
            "b1c": b1c,
            "b2c": b2c,
        })
    return in_maps


def kernel(h, w, src, dst, W1, b1, W2, b2):
    global LAST_EXEC_NS
    h = np.asarray(h, dtype=np.float32)
    w = np.asarray(w, dtype=np.float32)
    src = np.asarray(src)
    dst = np.asarray(dst)
    W1 = np.asarray(W1, dtype=np.float32)
    b1 = np.asarray(b1, dtype=np.float32)
    W2 = np.asarray(W2, dtype=np.float32)
    b2 = np.asarray(b2, dtype=np.float32)

    in_maps = _host_prep(h, w, src, dst, W1, b1, W2, b2)
    nc = _get_nc()
    trace = os.environ.get("GNN_PROFILE") == "1" and _enable_profile_hook()
    res = bass_utils.run_bass_kernel_spmd(
        nc, in_maps, core_ids=list(range(N_CORES)), trace=trace)
    LAST_EXEC_NS = res.exec_time_ns

    out = np.concatenate(
        [res.results[c]["outT"].T for c in range(N_CORES)], axis=0)
    return out.astype(np.float32)


# revision 11
# speedup vs baseline: 1.9443x; 1.6361x over previous
"""Trainium2 Bass kernel for a 2-layer edge-weighted GraphSAGE network.

Strategy (8 NeuronCores, dst-sharded):
  * Host converts the edge list (src, dst, w) into the dense row-normalized
    adjacency operator A[d, s] = sum_e w_e / max(deg_d, 1), so each layer's
    weighted segment-mean becomes a dense matmul h_N = A @ h.
  * Nodes (rows of A) are sharded across the 8 cores: core c owns dst range
    [1250c, 1250(c+1)).  A^T is stored fp8e4m3 scaled by 64 (keeps entries in
    the fp8 normal range; the 1/64 is folded into the PSUM->SBUF copy), so
    the whole 12.5MB per-core shard is loaded once and stays resident in
    SBUF — layer 2 re-reads it for free.
  * Aggregations run transposed on the TensorEngine: features on PSUM
    partitions, local dst nodes on the free axis; fp16 stationary x fp8
    moving, f32 accumulate.
  * Layer-1 output x is produced twice: fp16 for the local linear path and
    fp8 for aggregation; the fp8 copy is PE-transposed and AllGathered in
    two column-halves so the second half's transfer hides under layer-2
    compute.  A tiny warm-up collective at kernel start absorbs the one-time
    collective rendezvous / launch-skew cost.
  * Measured end-to-end relative error vs the f32 reference: ~4e-3.
"""

import os
import sys
import types

sys.path.insert(0, "/opt/trn_rl_repo")

import numpy as np

import concourse.bacc as bacc
import concourse.tile as tile
from concourse import mybir
from concourse import bass_utils
from concourse.masks import make_identity

N_NODES = 10000
N_EDGES = 640000
D_IN, D_HID, D_OUT = 128, 256, 64
N_CORES = 8
P = 128
NB = N_NODES // N_CORES          # 1250 local dst nodes per core
KR = 79                          # real src k-blocks (ceil(10000/128))
KB = 80                          # padded to a multiple of the quad size
KQ = KB // 2                     # A^T stream pairs
NPAD = KB * P
ASCALE = 64.0                    # fp8 pre-scale on A (undone in ACT copies)
F8 = mybir.dt.float8e4
F16 = mybir.dt.float16
F32 = mybir.dt.float32

# free-axis chunks of the local dst range (PSUM bank = 512 f32)
N_CHUNKS = [(0, 512), (512, 1024), (1024, NB)]
DST_BLOCKS = [(b * P, min((b + 1) * P, NB)) for b in range((NB + P - 1) // P)]
XG = 8                           # x k-blocks per batched load

_compiled_nc = None
LAST_EXEC_NS = None


def _build_nc():
    nc = bacc.Bacc("TRN2", target_bir_lowering=False, debug=False,
                   num_devices=N_CORES)

    as_d = nc.dram_tensor("as8", [KQ, P, 2 * NB], F8, kind="ExternalInput")
    hs_d = nc.dram_tensor("hsb", [P, KB * D_IN], F16, kind="ExternalInput")
    ht_d = nc.dram_tensor("ht", [D_IN, NB], F16, kind="ExternalInput")
    w1_d = nc.dram_tensor("w1", [2 * D_IN, D_HID], F16, kind="ExternalInput")
    w2_d = nc.dram_tensor("w2", [2 * D_HID, D_OUT], F16, kind="ExternalInput")
    b1_d = nc.dram_tensor("b1c", [P, 2], F32, kind="ExternalInput")
    b2_d = nc.dram_tensor("b2c", [D_OUT, 1], F32, kind="ExternalInput")
    out_d = nc.dram_tensor("outT", [D_OUT, NB], F32, kind="ExternalOutput")

    with tile.TileContext(nc) as tc:
        with (
            tc.tile_pool(name="const", bufs=1) as cpool,
            tc.tile_pool(name="acache", bufs=1) as acpool,
            tc.tile_pool(name="work", bufs=1) as wpool,
            tc.tile_pool(name="xstream", bufs=1) as xpool,
            tc.tile_pool(name="dram", bufs=1, space="DRAM") as dpool,
        ):
            # ---- warm-up collective: absorbs the one-time collective init /
            # cross-core launch-skew rendezvous in parallel with layer 1.
            warm_sb = cpool.tile([1, 16], F16)
            nc.vector.memset(warm_sb[:], 0.0)
            warm_in = dpool.tile([1, 16], F16)
            warm_out = dpool.tile([N_CORES, 16], F16, addr_space="Shared")
            nc.gpsimd.dma_start(out=warm_in[:], in_=warm_sb[:])
            nc.gpsimd.collective_compute(
                "AllGather", mybir.AluOpType.bypass,
                replica_groups=[list(range(N_CORES))],
                ins=[warm_in.opt()], outs=[warm_out.opt()])

            # ---- resident loads: h k-blocks (scalar ring) + full A^T (both) --
            hsb = cpool.tile([P, KB * D_IN], F16)
            HC = KB * D_IN // 4
            for j in range(4):
                nc.scalar.dma_start(out=hsb[:, j * HC:(j + 1) * HC],
                                    in_=hs_d[:, j * HC:(j + 1) * HC])
            acq = [acpool.tile([P, 2 * NB], F8, name=f"acq{q}")
                   for q in range(KQ)]
            for q in range(KQ):
                eng = nc.sync if q % 2 == 0 else nc.scalar
                eng.dma_start(out=acq[q][:], in_=as_d[q])

            def art(k, n0, n1):
                return acq[k // 2][:, (k % 2) * NB + n0:(k % 2) * NB + n1]

            hts = cpool.tile([P, NB], F16)
            nc.scalar.dma_start(out=hts[:], in_=ht_d[:])
            w1s = cpool.tile([P, 2 * D_HID], F16)
            for k in range(2):
                nc.scalar.dma_start(out=w1s[:, k * D_HID:(k + 1) * D_HID],
                                    in_=w1_d[k * P:(k + 1) * P, :])
            w2s = cpool.tile([P, 4 * D_OUT], F16)
            for k in range(4):
                nc.scalar.dma_start(out=w2s[:, k * D_OUT:(k + 1) * D_OUT],
                                    in_=w2_d[k * P:(k + 1) * P, :])
            b1s = cpool.tile([P, 2], F32)
            nc.scalar.dma_start(out=b1s[:], in_=b1_d[:])
            b2s = cpool.tile([D_OUT, 1], F32)
            nc.scalar.dma_start(out=b2s[:], in_=b2_d[:])
            ident = cpool.tile([P, P], F16)
            make_identity(nc, ident[:])

            hNT = wpool.tile([P, NB], F16)
            xT = [wpool.tile([P, NB], F16, name=f"xT{m}") for m in range(2)]
            xNT = [wpool.tile([P, NB], F16, name=f"xNT{m}") for m in range(2)]
            xloc8 = wpool.tile([P, len(DST_BLOCKS) * D_HID], F8)
            outsb = wpool.tile([D_OUT, NB], F32)

            # ---- layer 1 aggregation: hN^T = (1/64) sum_k hk^T . As_k -------
            with tc.tile_pool(name="ps1", bufs=1, space="PSUM") as ps1:
                hN_ps = ps1.tile([P, NB], F32, space="PSUM")
                for k in range(KR):
                    for (n0, n1) in N_CHUNKS:
                        nc.tensor.matmul(out=hN_ps[:, n0:n1],
                                         lhsT=hsb[:, k * D_IN:(k + 1) * D_IN],
                                         rhs=art(k, n0, n1),
                                         start=(k == 0), stop=(k == KR - 1))
                nc.scalar.activation(out=hNT[:], in_=hN_ps[:],
                                     func=mybir.ActivationFunctionType.Copy,
                                     scale=1.0 / ASCALE)

            # ---- layer 1 linear: x^T = relu(W1^T . [h; hN]^T + b1) ----------
            # x is produced twice: fp16 for the local linear path, fp8 for
            # the aggregation/all-gather path.
            cat1 = [hts, hNT]
            with tc.tile_pool(name="ps2", bufs=1, space="PSUM") as ps2:
                y_ps = [ps2.tile([P, NB], F32, space="PSUM", name=f"y_ps{m}")
                        for m in range(2)]
                for m in range(2):
                    for (n0, n1) in N_CHUNKS:
                        for k in range(2):
                            nc.tensor.matmul(
                                out=y_ps[m][:, n0:n1],
                                lhsT=w1s[:, k * D_HID + m * P: k * D_HID + (m + 1) * P],
                                rhs=cat1[k][:, n0:n1],
                                start=(k == 0), stop=(k == 1))
                for m in range(2):
                    nc.scalar.activation(out=xT[m][:], in_=y_ps[m][:],
                                         func=mybir.ActivationFunctionType.Relu,
                                         bias=b1s[:, m:m + 1])

            # ---- transpose x8^T -> x8 (row-major local shard) ---------------
            with tc.tile_pool(name="ps3", bufs=2, space="PSUM") as ps3:
                for m in range(2):          # m-major: half 0 fully first
                    for b, (d0, d1) in enumerate(DST_BLOCKS):
                        bw = d1 - d0
                        tps = ps3.tile([P, P], F16, space="PSUM", name="tps")
                        nc.tensor.transpose(out=tps[:bw, :],
                                            in_=xT[m][:, d0:d1],
                                            identity=ident[:])
                        nc.vector.tensor_copy(
                            out=xloc8[:bw, b * D_HID + m * P: b * D_HID + (m + 1) * P],
                            in_=tps[:bw, :])

            # ---- all-gather x8 across cores, split in column halves ---------
            ag_in = [dpool.tile([NB, P], F8, name=f"ag_in{m}") for m in range(2)]
            ag_out = [dpool.tile([N_NODES, P], F8, addr_space="Shared",
                                 name=f"ag_out{m}") for m in range(2)]
            for m in range(2):
                for b, (d0, d1) in enumerate(DST_BLOCKS):
                    bw = d1 - d0
                    nc.scalar.dma_start(
                        out=ag_in[m][d0:d1, :],
                        in_=xloc8[:bw, b * D_HID + m * P: b * D_HID + (m + 1) * P])
            for m in range(2):
                nc.gpsimd.collective_compute(
                    "AllGather", mybir.AluOpType.bypass,
                    replica_groups=[list(range(N_CORES))],
                    ins=[ag_in[m].opt()], outs=[ag_out[m].opt()])

            # ---- layer 2 aggregation: xN^T = (1/64) sum_k xk^T . As_k -------
            # Column-half m runs as soon as its all-gather lands; half 0's
            # compute hides half 1's transfer.
            with tc.tile_pool(name="ps4", bufs=1, space="PSUM") as ps4:
                xN_ps = [ps4.tile([P, NB], F32, space="PSUM", name=f"xN_ps{m}")
                         for m in range(2)]
                for m in range(2):
                    for g in range(0, KB, XG):
                        gk = min(XG, KB - g)
                        xq = xpool.tile([P, XG * P], F8, name=f"xq{m}", bufs=3)
                        full = min((g + gk) * P, (KR - 1) * P) - g * P
                        nc.scalar.dma_start(
                            out=xq[:, :full].rearrange("p (k f) -> p k f", f=P),
                            in_=ag_out[m][g * P: g * P + full, :]
                                .rearrange("(k p) f -> p k f", p=P))
                        if g + gk == KB:  # ragged block 78 + zero pad block 79
                            nc.vector.memset(xq[:, (gk - 2) * P:], 0.0)
                            tail = N_NODES - (KR - 1) * P
                            nc.scalar.dma_start(
                                out=xq[:tail, (gk - 2) * P: (gk - 1) * P],
                                in_=ag_out[m][(KR - 1) * P:, :])
                        # fp8 DoubleRow: contract k-block pairs, 2 MACs/cycle
                        for kp in range(g // 2, (g + gk) // 2):
                            kk2 = kp * 2 - g
                            lhs_pair = xq[:, kk2 * P:(kk2 + 2) * P] \
                                .rearrange("p (two f) -> p two f", two=2)
                            rhs_pair = acq[kp][:] \
                                .rearrange("p (two d) -> p two d", two=2)
                            for (n0, n1) in N_CHUNKS:
                                nc.tensor.matmul(
                                    out=xN_ps[m][:, n0:n1],
                                    lhsT=lhs_pair,
                                    rhs=rhs_pair[:, :, n0:n1],
                                    perf_mode=mybir.MatmulPerfMode.DoubleRow,
                                    start=(kp == 0), stop=(kp == KB // 2 - 1))
                    nc.scalar.activation(out=xNT[m][:], in_=xN_ps[m][:],
                                         func=mybir.ActivationFunctionType.Copy,
                                         scale=1.0 / ASCALE)

            # ---- layer 2 linear: out^T = W2^T . [x; xN]^T + b2 --------------
            cat2 = [xT[0], xT[1], xNT[0], xNT[1]]
            with tc.tile_pool(name="ps5", bufs=1, space="PSUM") as ps5:
                o_ps = ps5.tile([D_OUT, NB], F32, space="PSUM")
                for (n0, n1) in N_CHUNKS:
                    for k in range(4):
                        nc.tensor.matmul(
                            out=o_ps[:, n0:n1],
                            lhsT=w2s[:, k * D_OUT:(k + 1) * D_OUT],
                            rhs=cat2[k][:, n0:n1],
                            start=(k == 0), stop=(k == 3))
                nc.scalar.activation(out=outsb[:], in_=o_ps[:],
                                     func=mybir.ActivationFunctionType.Identity,
                                     bias=b2s[:, 0:1])
            nc.sync.dma_start(out=out_d[:], in_=outsb[:])

    nc.compile()
    return nc


def _get_nc():
    global _compiled_nc
    if _compiled_nc is None:
        _compiled_nc = _build_nc()
    return _compiled_nc


def _enable_profile_hook():
    """Register the NTFF profiling hook that trn_boot skips when the image's
    antenv lacks axon_hooks (profiling only; used when GNN_PROFILE=1)."""
    try:
        import antenv
        if "antenv.axon_hooks" not in sys.modules:
            mod = types.ModuleType("antenv.axon_hooks")
            _h = [None]
            mod.set_axon_ntff_profile_hook = lambda hook: _h.__setitem__(0, hook)
            mod.get_axon_ntff_profile_hook = lambda: _h[0]
            sys.modules["antenv.axon_hooks"] = mod
            antenv.axon_hooks = mod
        from trn_agent_boot.trn_boot import _ntff_profile_via_ctypes
        hook = _ntff_profile_via_ctypes("/opt/axon/libaxon_pjrt.so")
        if hook is not None:
            sys.modules["antenv.axon_hooks"].set_axon_ntff_profile_hook(hook)
            return True
    except Exception:
        pass
    return False


def _host_prep(h, w, src, dst, W1, b1, W2, b2):
    import ml_dtypes
    import scipy.sparse as sp
    deg = np.bincount(dst, minlength=N_NODES).astype(np.float32)
    w_norm = (w[:, 0] * (ASCALE / np.maximum(deg, 1.0)[dst])).astype(np.float32)
    # AT[s, d] = sum of scaled w_norm over edges (s -> d): 64*A^T
    AT = sp.coo_matrix((w_norm, (src, dst)), shape=(N_NODES, N_NODES)).toarray()
    AT8 = np.zeros((NPAD, N_NODES), dtype=ml_dtypes.float8_e4m3)
    AT8[:N_NODES] = AT
    hp = np.zeros((NPAD, D_IN), dtype=np.float16)
    hp[:N_NODES] = h
    # hsb[p, k*128+f] = h[k*128+p, f] (SBUF layout, contiguous per partition)
    hsb = np.ascontiguousarray(
        hp.reshape(KB, P, D_IN).transpose(1, 0, 2).reshape(P, KB * D_IN))

    w1c = W1.astype(np.float16)
    w2c = W2.astype(np.float16)
    b1c = np.ascontiguousarray(b1.reshape(2, P).T)
    b2c = b2.reshape(D_OUT, 1)

    in_maps = []
    for c in range(N_CORES):
        sl = slice(c * NB, (c + 1) * NB)
        # as8[q, p, j*NB+d] = AT8[(2q+j)*128+p, c*NB+d] (pair-interleaved)
        as8 = np.ascontiguousarray(
            AT8[:, sl].reshape(KQ, 2, P, NB).transpose(0, 2, 1, 3)
            .reshape(KQ, P, 2 * NB))
        in_maps.append({
            "as8": as8,
            "hsb": hsb,
            "ht": np.ascontiguousarray(h[sl].T.astype(np.float16)),
            "w1": w1c,
            "w2": w2c,
            "b1c": b1c,
            "b2c": b2c,
        })
    return in_maps


def kernel(h, w, src, dst, W1, b1, W2, b2):
    global LAST_EXEC_NS
    h = np.asarray(h, dtype=np.float32)
    w = np.asarray(w, dtype=np.float32)
    src = np.asarray(src)
    dst = np.asarray(dst)
    W1 = np.asarray(W1, dtype=np.float32)
    b1 = np.asarray(b1, dtype=np.float32)
    W2 = np.asarray(W2, dtype=np.float32)
    b2 = np.asarray(b2, dtype=np.float32)

    in_maps = _host_prep(h, w, src, dst, W1, b1, W2, b2)
    nc = _get_nc()
    trace = os.environ.get("GNN_PROFILE") == "1" and _enable_profile_hook()
    res = bass_utils.run_bass_kernel_spmd(
        nc, in_maps, core_ids=list(range(N_CORES)), trace=trace)
    LAST_EXEC_NS = res.exec_time_ns

    out = np.concatenate(
        [res.results[c]["outT"].T for c in range(N_CORES)], axis=0)
    return out.astype(np.float32)
